# revision 3
# baseline (speedup 1.0000x reference)
"""Multi-head attention (qkv pointwise-conv projection + softmax attention)
on 8 Trainium2 NeuronCores.

Problem shapes (hardcoded):
    x:     [B=4, D=512, L=2048] f32
    w_qkv: [3*D=1536, D=512]    f32
    out:   [B, D, L]            f32

Sharding: 2 cores per batch element; each core owns 4 of the 8 heads
(tensor-parallel on the qkv output channels). Core c -> batch c//2,
head group c%2 (heads 4*(c%2) .. 4*(c%2)+3).

Per-core kernel (all in bf16 compute, f32 accumulate):
    Q/K proj:  q[o,l] = sum_d w[o,d] x[d,l]   (layout [head_dim, L])
    V proj  :  vT[l,o]                          (layout [L, head_dim])
               vT stored per head with a fused ones-column -> attn@[v|1]
               yields both the weighted values and the softmax denominator.
    scores  :  St[j,i] = sum_d k[d,j] q[d,i]  (two heads packed in the
               128-row PE array via row tiling: head0 partitions 0-63,
               head1 partitions 64-127)
    softmax :  exp on ScalarE (scale folded into the activation), no max
               subtraction (scores are O(1) by construction)
    attn@v  :  O[d(+den),i] accumulated over j blocks in PSUM
    norm    :  O[d,i] * broadcast(1/den[i])  (broadcast via K=1 matmul)

ScalarE is the bottleneck (128 exps x ~1.11us = 142us of 180us): the
schedule aims to keep the exp stream gapless. Startup: input DMAs go out
on three parallel DGE queues (SP + Act HWDGE, Pool SWDGE) with x chunk 0
split in halves and w_qkv split in per-(pair,q|k) sections so the first
scores land ~4us earlier. Fillers: pair-1 projections run inside pair-1
blocks (each block has ~4us of PE slack vs the exp cadence), k0 column
groups are front-loaded in block 0, vt is just-in-time. Tail: the final
block normalizes via ScalarE den-copies (idle after the last exp),
GpSimd does one of the two multiplies, and the two output DMAs use
separate queues.
"""

import os
import numpy as np

B, D, L, H = 4, 512, 2048, 8
HD = D // H  # 64
N_CORES = 8
SCALE = float(D) ** -0.5

# module-level knobs for test.py; harness uses defaults
TRACE = False
LAST_RESULTS = None

_COMPILED = {}


def _build_nc():
    from contextlib import ExitStack

    import concourse.bass as bass
    import concourse.mybir as mybir
    import concourse.tile as tile
    from concourse.bacc import Bacc

    F32 = mybir.dt.float32
    BF16 = mybir.dt.bfloat16
    Exp = mybir.ActivationFunctionType.Exp

    # Bacc (not plain Bass): its finalize() runs the legalization passes that
    # split multi-wait matmuls (walrus MM struct supports only 1 sync wait).
    nc = Bacc("TRN2", target_bir_lowering=False, debug=False)
    # host pre-permuted layouts -> fully contiguous DMA descriptors
    # x: [p, lc, dc, l'] where d = dc*128+p, l = lc*512+l'
    x_d = nc.dram_tensor("x", [128, 4, 4, 512], BF16, kind="ExternalInput")
    # wT split in 4 sections s = 2*pair + (0 q | 1 k): [s, p, dc, o]
    wqk_d = nc.dram_tensor("wqkT", [4, 128, 4, 128], BF16, kind="ExternalInput")
    wv_d = nc.dram_tensor("wvT", [128, 4, 256], BF16, kind="ExternalInput")
    out_d = nc.dram_tensor("out", [256, L], F32, kind="ExternalOutput")

    NJB = L // 128  # 16 key blocks
    NIC = L // 512  # 4 query chunks

    with ExitStack() as ctx:
        tc = ctx.enter_context(tile.TileContext(nc))
        const = ctx.enter_context(tc.tile_pool(name="const", bufs=1))
        qkp = ctx.enter_context(tc.tile_pool(name="qkp", bufs=1))
        vtp = ctx.enter_context(tc.tile_pool(name="vtp", bufs=1))
        sx = ctx.enter_context(tc.tile_pool(name="sx", bufs=4))
        nrm = ctx.enter_context(tc.tile_pool(name="nrm", bufs=4))
        outp = ctx.enter_context(tc.tile_pool(name="outp", bufs=4))
        drp = ctx.enter_context(tc.tile_pool(name="drp", bufs=4, space="DRAM"))
        ps_st = ctx.enter_context(tc.tile_pool(name="ps_st", bufs=2, space="PSUM"))
        ps_o = ctx.enter_context(tc.tile_pool(name="ps_o", bufs=4, space="PSUM"))

        # ---- PE warmup + load inputs ----
        # a few matmuls on zeros keep the PE busy while the first input DMAs
        # land, so the HAM clock gate starts opening (1.2 -> 2.4 GHz).
        scr_sb = const.tile([128, 512], BF16, tag="scr")
        nc.vector.memset(scr_sb[:], 0.0)
        warm_ps = ps_st.tile([128, 1024], F32, tag="st", name="warm")
        for _ in range(3):
            nc.tensor.matmul(warm_ps[:, 0:512], scr_sb[:, 0:128], scr_sb[:])

        wqk_sb = const.tile([128, 4, 4, 128], BF16, tag="wqk")
        wv_sb = const.tile([128, 4, 256], BF16, tag="wv")
        x_sb = const.tile([128, 4, 4, 512], BF16, tag="x")
        # three parallel DGE queues, ordered so the first projection group
        # (wqk sections 0-1 + x chunk 0) is resident ASAP
        nc.sync.dma_start(out=x_sb[:, 0, 0:2, :], in_=x_d[:, 0, 0:2, :])
        nc.scalar.dma_start(out=x_sb[:, 0, 2:4, :], in_=x_d[:, 0, 2:4, :])
        for s in range(4):
            nc.gpsimd.dma_start(out=wqk_sb[:, s, :, :], in_=wqk_d[s])
        nc.sync.dma_start(out=x_sb[:, 1, :, :], in_=x_d[:, 1, :, :])
        nc.scalar.dma_start(out=wv_sb[:], in_=wv_d[:])
        nc.sync.dma_start(out=x_sb[:, 2, :, :], in_=x_d[:, 2, :, :])
        nc.scalar.dma_start(out=x_sb[:, 3, :, :], in_=x_d[:, 3, :, :])
        ones_sb = const.tile([1, 64], F32, tag="ones")
        nc.vector.memset(ones_sb[:], 1.0)

        q_sb = [qkp.tile([128, L], BF16, tag=f"q{p}", name=f"q{p}") for p in range(2)]
        k_sb = [qkp.tile([128, L], BF16, tag=f"k{p}", name=f"k{p}") for p in range(2)]
        vt_sb = [vtp.tile([128, 4, 65], BF16, tag=f"vt{jb}", name=f"vt{jb}") for jb in range(NJB)]

        # Projection groups run in 1-bank [128,512] PSUM tiles from the shared
        # "o" pool so they never contend with the exp-feeding st pipeline.
        def g_qk(p, qk, lc):
            # one 512-wide column group of the Q (qk=0) or K (qk=1)
            # projection for head-pair p
            def f():
                dst = q_sb[p] if qk == 0 else k_sb[p]
                ps = ps_o.tile([128, 512], F32, tag="o", name="projg")
                for dc in range(4):
                    nc.tensor.matmul(
                        ps[:],
                        wqk_sb[:, 2 * p + qk, dc, :],
                        x_sb[:, lc, dc, :],
                        start=(dc == 0),
                        stop=(dc == 3),
                    )
                nc.vector.tensor_copy(dst[:, lc * 512 : (lc + 1) * 512], ps[:])

            return f

        def g_vt(jb):
            def f():
                nc.vector.memset(vt_sb[jb][:, :, 64:65], 1.0)
                ps = ps_o.tile([128, 512], F32, tag="o", name="projv")
                for dc in range(4):
                    nc.tensor.matmul(
                        ps[:, 0:256],
                        x_sb[:, jb // 4, dc, (jb % 4) * 128 : (jb % 4 + 1) * 128],
                        wv_sb[:, dc, :],
                        start=(dc == 0),
                        stop=(dc == 3),
                    )
                nc.vector.tensor_copy(
                    vt_sb[jb][:, :, 0:64],
                    ps[:, 0:256].rearrange("par (h e) -> par h e", e=64),
                )

            return f

        def attn_block(p, ic, fillers=(), last=False):
            # scores+softmax+attn@v for head pair p, query chunk ic (512 wide)
            # fillers: {jb: [callables]} — projection groups interleaved into
            # the loop to fill PE slack without starving ScalarE
            # last: tail-optimized normalization (ScalarE den copies, GpSimd
            # mul, parallel out-DMA queues, PE broadcast)
            fillers = dict(fillers)
            i0 = ic * 512

            def st_mms(jb):
                # St[j, i] for both heads of the pair, row-packed in the PE
                st = ps_st.tile([128, 1024], F32, tag="st")
                for hp in range(2):
                    nc.tensor.matmul(
                        st[:, hp * 512 : (hp + 1) * 512],
                        k_sb[p][hp * 64 : (hp + 1) * 64, jb * 128 : (jb + 1) * 128],
                        q_sb[p][hp * 64 : (hp + 1) * 64, i0 : i0 + 512],
                        start=True,
                        stop=True,
                    )
                return st

            o_ps = [ps_o.tile([65, 512], F32, tag="o", name="o_acc") for _ in range(2)]
            st_cur = st_mms(0)
            for jb in range(NJB):
                se = sx.tile([128, 1024], BF16, tag="se")
                nc.scalar.activation(se[:], st_cur[:], Exp, scale=SCALE)
                if jb + 1 < NJB:
                    st_cur = st_mms(jb + 1)
                for f in fillers.get(jb, ()):
                    f()
                for hp in range(2):
                    nc.tensor.matmul(
                        o_ps[hp][:],
                        vt_sb[jb][:, 2 * p + hp, :],
                        se[:, hp * 512 : (hp + 1) * 512],
                        start=(jb == 0),
                        stop=(jb == NJB - 1),
                    )

            if not last:
                # normalize and write out: 1/den on DVE (fast approx),
                # broadcast the row across 64 partitions via a DRAM bounce.
                for hp in range(2):
                    hh = 2 * p + hp
                    o = o_ps[hp]
                    den_sb = nrm.tile([1, 512], F32, tag="den")
                    nc.vector.tensor_copy(den_sb[:], o[64:65, :])
                    recip = nrm.tile([1, 512], F32, tag="recip")
                    # NB: approx-recip reads garbage from PSUM on HW; SBUF in only
                    nc.vector.reciprocal_approx_fast(out=recip[:], in_=den_sb[:])
                    rbc = nrm.tile([64, 512], F32, tag="rbc")
                    dbounce = drp.tile([1, 512], F32, tag="db", name="db")
                    nc.sync.dma_start(out=dbounce[:], in_=recip[:])
                    nc.sync.dma_start(
                        out=rbc[:],
                        in_=bass.AP(
                            tensor=dbounce.tensor,
                            offset=dbounce.offset,
                            ap=[[0, 64], [1, 512]],
                        ),
                    )
                    ot = outp.tile([64, 512], F32, tag="ot")
                    nc.vector.tensor_mul(ot[:], o[0:64, :], rbc[:])
                    nc.sync.dma_start(
                        out=out_d[hh * 64 : (hh + 1) * 64, i0 : i0 + 512], in_=ot[:]
                    )
                return

            # tail-optimized final normalization: den + broadcast copies on the
            # (now idle) ScalarE, reciprocals + multiplies on DVE, broadcast
            # matmul on the PE (short latency), out-DMAs on separate queues.
            # (GpSimd can't touch PSUM, so it stays out of this chain.)
            den_sb = [nrm.tile([1, 512], F32, tag="den", name=f"den{hp}") for hp in range(2)]
            recip = [nrm.tile([1, 512], F32, tag="recip", name=f"recip{hp}") for hp in range(2)]
            rbc = [nrm.tile([64, 512], F32, tag="rbc", name=f"rbc{hp}") for hp in range(2)]
            bc_ps = [ps_st.tile([128, 1024], F32, tag="st", name=f"bc{hp}") for hp in range(2)]
            ot = [outp.tile([64, 512], F32, tag="ot", name=f"ot{hp}") for hp in range(2)]
            for hp in range(2):
                nc.scalar.copy(den_sb[hp][:], o_ps[hp][64:65, :])
                nc.vector.reciprocal_approx_fast(out=recip[hp][:], in_=den_sb[hp][:])
                nc.tensor.matmul(
                    bc_ps[hp][0:64, 0:512], ones_sb[:], recip[hp][:],
                    start=True, stop=True,
                )
                nc.scalar.copy(rbc[hp][:], bc_ps[hp][0:64, 0:512])
            nc.vector.tensor_mul(ot[0][:], o_ps[0][0:64, :], rbc[0][:])
            nc.vector.tensor_mul(ot[1][:], o_ps[1][0:64, :], rbc[1][:])
            hh0 = 2 * p
            nc.sync.dma_start(
                out=out_d[hh0 * 64 : (hh0 + 1) * 64, i0 : i0 + 512], in_=ot[0][:]
            )
            nc.scalar.dma_start(
                out=out_d[(hh0 + 1) * 64 : (hh0 + 2) * 64, i0 : i0 + 512], in_=ot[1][:]
            )

        # prologue: just enough projection for the first scores (q0/k0 column
        # group 0); everything else is interleaved as fillers.
        # Constraints: vt[j] before av(j) of block (0,0); k[p] group m before
        # st(4m) of the first block using pair p; q[p] group lc before block
        # (p,lc).
        g_qk(0, 0, 0)()
        g_qk(0, 1, 0)()
        attn_block(0, 0, {
            0: [g_vt(0), g_qk(0, 1, 1)],
            1: [g_vt(1), g_vt(2)],
            2: [g_qk(0, 1, 2), g_vt(3)],
            3: [g_vt(4)],
            4: [g_vt(5)],
            5: [g_qk(0, 1, 3), g_vt(6)],
            6: [g_vt(7)],
            7: [g_vt(8)],
            8: [g_vt(9)],
            9: [g_vt(10)],
            10: [g_qk(0, 0, 1), g_vt(11)],
            11: [g_vt(12)],
            12: [g_vt(13)],
            13: [g_vt(14)],
            14: [g_vt(15)],
        })
        attn_block(0, 1, {2: [g_qk(0, 0, 2)]})
        attn_block(0, 2, {2: [g_qk(0, 0, 3)]})
        attn_block(0, 3, {2: [g_qk(1, 1, 0)], 8: [g_qk(1, 0, 0)]})
        attn_block(1, 0, {
            0: [g_qk(1, 1, 1)],
            3: [g_qk(1, 1, 2)],
            6: [g_qk(1, 1, 3)],
            10: [g_qk(1, 0, 1)],
        })
        attn_block(1, 1, {2: [g_qk(1, 0, 2)]})
        attn_block(1, 2, {2: [g_qk(1, 0, 3)]})
        attn_block(1, 3, last=True)

    nc.finalize()
    return nc


def _get_nc():
    if "nc" not in _COMPILED:
        _COMPILED["nc"] = _build_nc()
    return _COMPILED["nc"]


def _prep_inputs(x, w_qkv):
    """Per-core input maps (host-side sharding)."""
    import ml_dtypes

    bf16 = ml_dtypes.bfloat16
    in_maps = []
    for c in range(N_CORES):
        b, g = c // 2, c % 2
        # x[b] [512, 2048] -> [p, lc, dc, l'] so every DMA descriptor is a
        # 4KB contiguous run
        xb = np.ascontiguousarray(
            x[b].reshape(4, 128, 4, 512).transpose(1, 2, 0, 3)
        ).astype(bf16)
        # w rows for this head group, transposed then laid out in 4 sections
        # s = 2*pair + (0 q | 1 k): [s, p, dc, o]; v separate — all DMAs are
        # fully contiguous
        wq_rows = w_qkv[256 * g : 256 * (g + 1), :]  # [256, 512]
        wk_rows = w_qkv[512 + 256 * g : 512 + 256 * (g + 1), :]
        wv_rows = w_qkv[1024 + 256 * g : 1024 + 256 * (g + 1), :]
        # section s covers head pair p = s//2, q (s%2==0) or k: 128 columns of
        # wT = rows 128*p..128*(p+1) of the q|k block
        secs = []
        for p in range(2):
            for rows in (wq_rows, wk_rows):
                sec = rows[128 * p : 128 * (p + 1), :].T  # [512(d), 128(o)]
                secs.append(sec.reshape(4, 128, 128).transpose(1, 0, 2))
        wqkT = np.ascontiguousarray(np.stack(secs, axis=0)).astype(bf16)
        wvT = np.ascontiguousarray(
            wv_rows.T.reshape(4, 128, 256).transpose(1, 0, 2)
        ).astype(bf16)
        in_maps.append({"x": xb, "wqkT": wqkT, "wvT": wvT})
    return in_maps


def kernel(x, w_qkv):
    global LAST_RESULTS
    from concourse.bass_utils import run_bass_kernel_spmd

    nc = _get_nc()
    in_maps = _prep_inputs(np.asarray(x), np.asarray(w_qkv))
    res = run_bass_kernel_spmd(
        nc, in_maps, core_ids=list(range(N_CORES)), trace=TRACE
    )
    LAST_RESULTS = res
    out = np.empty((B, D, L), dtype=np.float32)
    for c in range(N_CORES):
        b, g = c // 2, c % 2
        out[b, 256 * g : 256 * (g + 1), :] = res.results[c]["out"]
    return out


# revision 8
# speedup vs baseline: 1.0003x; 1.0003x over previous
"""Multi-head attention (qkv pointwise-conv projection + softmax attention)
on 8 Trainium2 NeuronCores.

Problem shapes (hardcoded):
    x:     [B=4, D=512, L=2048] f32
    w_qkv: [3*D=1536, D=512]    f32
    out:   [B, D, L]            f32

Sharding: 2 cores per batch element; each core owns 4 of the 8 heads
(tensor-parallel on the qkv output channels). Core c -> batch c//2,
head group c%2 (heads 4*(c%2) .. 4*(c%2)+3).

Per-core kernel (all in bf16 compute, f32 accumulate):
    Q/K proj:  q[o,l] = sum_d w[o,d] x[d,l]   (layout [head_dim, L])
    V proj  :  vT[l,o]                          (layout [L, head_dim])
               vT stored per head with a fused ones-column -> attn@[v|1]
               yields both the weighted values and the softmax denominator.
    scores  :  St[j,i] = sum_d k[d,j] q[d,i]  (two heads packed in the
               128-row PE array via row tiling: head0 partitions 0-63,
               head1 partitions 64-127)
    softmax :  exp on ScalarE (scale folded into the activation), no max
               subtraction (scores are O(1) by construction)
    attn@v  :  O[d(+den),i] accumulated over j blocks in PSUM
    norm    :  O[d,i] * broadcast(1/den[i])  (broadcast via K=1 matmul)

ScalarE is the bottleneck (128 exps x ~1.11us = 142us of 180us): the
schedule aims to keep the exp stream gapless. Startup: input DMAs go out
on three parallel DGE queues (SP + Act HWDGE, Pool SWDGE) with x chunk 0
split in halves and w_qkv split in per-(pair,q|k) sections so the first
scores land ~4us earlier. Fillers: pair-1 projections run inside pair-1
blocks (each block has ~4us of PE slack vs the exp cadence), k0 column
groups are front-loaded in block 0, vt is just-in-time. Tail: the final
block normalizes via ScalarE den-copies (idle after the last exp),
GpSimd does one of the two multiplies, and the two output DMAs use
separate queues.
"""

import os
import numpy as np

B, D, L, H = 4, 512, 2048, 8
HD = D // H  # 64
N_CORES = 8
SCALE = float(D) ** -0.5

# module-level knobs for test.py; harness uses defaults
TRACE = False
LAST_RESULTS = None

_COMPILED = {}


def _build_nc():
    from contextlib import ExitStack

    import concourse.bass as bass
    import concourse.mybir as mybir
    import concourse.tile as tile
    from concourse.bacc import Bacc

    F32 = mybir.dt.float32
    BF16 = mybir.dt.bfloat16
    Exp = mybir.ActivationFunctionType.Exp

    # Bacc (not plain Bass): its finalize() runs the legalization passes that
    # split multi-wait matmuls (walrus MM struct supports only 1 sync wait).
    nc = Bacc("TRN2", target_bir_lowering=False, debug=False)
    # host pre-permuted layouts -> fully contiguous DMA descriptors
    # x: [p, lc, dc, l'] where d = dc*128+p, l = lc*512+l'
    x_d = nc.dram_tensor("x", [128, 4, 4, 512], BF16, kind="ExternalInput")
    # wT split in 4 sections s = 2*pair + (0 q | 1 k): [s, p, dc, o]
    wqk_d = nc.dram_tensor("wqkT", [4, 128, 4, 128], BF16, kind="ExternalInput")
    wv_d = nc.dram_tensor("wvT", [128, 4, 256], BF16, kind="ExternalInput")
    out_d = nc.dram_tensor("out", [256, L], F32, kind="ExternalOutput")

    NJB = L // 128  # 16 key blocks
    NIC = L // 512  # 4 query chunks

    with ExitStack() as ctx:
        tc = ctx.enter_context(tile.TileContext(nc))
        const = ctx.enter_context(tc.tile_pool(name="const", bufs=1))
        qkp = ctx.enter_context(tc.tile_pool(name="qkp", bufs=1))
        vtp = ctx.enter_context(tc.tile_pool(name="vtp", bufs=1))
        sx = ctx.enter_context(tc.tile_pool(name="sx", bufs=6))
        nrm = ctx.enter_context(tc.tile_pool(name="nrm", bufs=4))
        outp = ctx.enter_context(tc.tile_pool(name="outp", bufs=4))
        drp = ctx.enter_context(tc.tile_pool(name="drp", bufs=4, space="DRAM"))
        ps_st = ctx.enter_context(tc.tile_pool(name="ps_st", bufs=2, space="PSUM"))
        ps_o = ctx.enter_context(tc.tile_pool(name="ps_o", bufs=4, space="PSUM"))

        # ---- PE warmup + load inputs ----
        # a few matmuls on zeros keep the PE busy while the first input DMAs
        # land, so the HAM clock gate starts opening (1.2 -> 2.4 GHz).
        scr_sb = const.tile([128, 512], BF16, tag="scr")
        nc.vector.memset(scr_sb[:], 0.0)
        warm_ps = ps_st.tile([128, 1024], F32, tag="st", name="warm")
        for _ in range(5):
            nc.tensor.matmul(warm_ps[:, 0:512], scr_sb[:, 0:128], scr_sb[:])

        wqk_sb = const.tile([128, 4, 4, 128], BF16, tag="wqk")
        wv_sb = const.tile([128, 4, 256], BF16, tag="wv")
        x_sb = const.tile([128, 4, 4, 512], BF16, tag="x")
        # Both HWDGE rings spread descriptors over all DMA engines, and each
        # ring drains its queue in issue order — so in-queue order IS the
        # priority: the critical first-projection set (wqk sections 0-1 +
        # x chunk 0, 0.75MB) goes first, bulk chunks queue behind it. The
        # single-engine SWDGE (gpsimd) ring only gets the latest-needed
        # pair-1 weight sections.
        nc.sync.dma_start(out=wqk_sb[:, 0, :, :], in_=wqk_d[0])
        nc.scalar.dma_start(out=wqk_sb[:, 1, :, :], in_=wqk_d[1])
        nc.sync.dma_start(out=x_sb[:, 0, 0:2, :], in_=x_d[:, 0, 0:2, :])
        nc.scalar.dma_start(out=x_sb[:, 0, 2:4, :], in_=x_d[:, 0, 2:4, :])
        nc.sync.dma_start(out=x_sb[:, 1, :, :], in_=x_d[:, 1, :, :])
        nc.scalar.dma_start(out=wv_sb[:], in_=wv_d[:])
        nc.sync.dma_start(out=x_sb[:, 2, :, :], in_=x_d[:, 2, :, :])
        nc.scalar.dma_start(out=x_sb[:, 3, :, :], in_=x_d[:, 3, :, :])
        nc.gpsimd.dma_start(out=wqk_sb[:, 2, :, :], in_=wqk_d[2])
        nc.gpsimd.dma_start(out=wqk_sb[:, 3, :, :], in_=wqk_d[3])
        ones_sb = const.tile([1, 64], F32, tag="ones")
        nc.vector.memset(ones_sb[:], 1.0)

        q_sb = [qkp.tile([128, L], BF16, tag=f"q{p}", name=f"q{p}") for p in range(2)]
        k_sb = [qkp.tile([128, L], BF16, tag=f"k{p}", name=f"k{p}") for p in range(2)]
        vt_sb = [vtp.tile([128, 4, 65], BF16, tag=f"vt{jb}", name=f"vt{jb}") for jb in range(NJB)]

        # Projection groups run in 1-bank [128,512] PSUM tiles from the shared
        # "o" pool so they never contend with the exp-feeding st pipeline.
        def g_qk(p, qk, lc):
            # one 512-wide column group of the Q (qk=0) or K (qk=1)
            # projection for head-pair p
            def f():
                dst = q_sb[p] if qk == 0 else k_sb[p]
                ps = ps_o.tile([128, 512], F32, tag="o", name="projg")
                for dc in range(4):
                    nc.tensor.matmul(
                        ps[:],
                        wqk_sb[:, 2 * p + qk, dc, :],
                        x_sb[:, lc, dc, :],
                        start=(dc == 0),
                        stop=(dc == 3),
                    )
                nc.vector.tensor_copy(dst[:, lc * 512 : (lc + 1) * 512], ps[:])

            return f

        def g_qk_split(p, qk, lc, dcs):
            # g_qk sub-split into per-iteration pieces (each <=2 matmuls,
            # ~426ns) so filler load never spikes a single exp interval.
            # Returns a list of callables; schedule them on consecutive jbs.
            state = {}

            def part(dc_list):
                def f():
                    if 0 in dc_list:
                        state["ps"] = ps_o.tile(
                            [128, 512], F32, tag="o", name="projg"
                        )
                    for dc in dc_list:
                        nc.tensor.matmul(
                            state["ps"][:],
                            wqk_sb[:, 2 * p + qk, dc, :],
                            x_sb[:, lc, dc, :],
                            start=(dc == 0),
                            stop=(dc == 3),
                        )
                    if 3 in dc_list:
                        dst = q_sb[p] if qk == 0 else k_sb[p]
                        nc.vector.tensor_copy(
                            dst[:, lc * 512 : (lc + 1) * 512], state["ps"][:]
                        )

                return f

            return [part(dcl) for dcl in dcs]

        def g_vt(jb):
            def f():
                nc.vector.memset(vt_sb[jb][:, :, 64:65], 1.0)
                ps = ps_o.tile([128, 512], F32, tag="o", name="projv")
                for dc in range(4):
                    nc.tensor.matmul(
                        ps[:, 0:256],
                        x_sb[:, jb // 4, dc, (jb % 4) * 128 : (jb % 4 + 1) * 128],
                        wv_sb[:, dc, :],
                        start=(dc == 0),
                        stop=(dc == 3),
                    )
                nc.vector.tensor_copy(
                    vt_sb[jb][:, :, 0:64],
                    ps[:, 0:256].rearrange("par (h e) -> par h e", e=64),
                )

            return f

        def attn_block(p, ic, fillers=(), last=False):
            # scores+softmax+attn@v for head pair p, query chunk ic (512 wide)
            # fillers: {jb: [callables]} — projection groups interleaved into
            # the loop to fill PE slack without starving ScalarE
            # last: tail-optimized normalization (ScalarE den copies, GpSimd
            # mul, parallel out-DMA queues, PE broadcast)
            fillers = dict(fillers)
            i0 = ic * 512

            def st_mms(jb):
                # St[j, i] for both heads of the pair, row-packed in the PE
                st = ps_st.tile([128, 1024], F32, tag="st")
                for hp in range(2):
                    nc.tensor.matmul(
                        st[:, hp * 512 : (hp + 1) * 512],
                        k_sb[p][hp * 64 : (hp + 1) * 64, jb * 128 : (jb + 1) * 128],
                        q_sb[p][hp * 64 : (hp + 1) * 64, i0 : i0 + 512],
                        start=True,
                        stop=True,
                    )
                return st

            o_ps = [ps_o.tile([65, 512], F32, tag="o", name="o_acc") for _ in range(2)]
            st_cur = st_mms(0)
            for jb in range(NJB):
                se = sx.tile([128, 1024], BF16, tag="se")
                nc.scalar.activation(se[:], st_cur[:], Exp, scale=SCALE)
                if jb + 1 < NJB:
                    st_cur = st_mms(jb + 1)
                for f in fillers.get(jb, ()):
                    f()
                for hp in range(2):
                    nc.tensor.matmul(
                        o_ps[hp][:],
                        vt_sb[jb][:, 2 * p + hp, :],
                        se[:, hp * 512 : (hp + 1) * 512],
                        start=(jb == 0),
                        stop=(jb == NJB - 1),
                    )

            if not last:
                # normalize and write out: 1/den on DVE (fast approx),
                # broadcast the row across 64 partitions via a DRAM bounce.
                for hp in range(2):
                    hh = 2 * p + hp
                    o = o_ps[hp]
                    den_sb = nrm.tile([1, 512], F32, tag="den")
                    nc.vector.tensor_copy(den_sb[:], o[64:65, :])
                    recip = nrm.tile([1, 512], F32, tag="recip")
                    # NB: approx-recip reads garbage from PSUM on HW; SBUF in only
                    nc.vector.reciprocal_approx_fast(out=recip[:], in_=den_sb[:])
                    rbc = nrm.tile([64, 512], F32, tag="rbc")
                    dbounce = drp.tile([1, 512], F32, tag="db", name="db")
                    nc.sync.dma_start(out=dbounce[:], in_=recip[:])
                    nc.sync.dma_start(
                        out=rbc[:],
                        in_=bass.AP(
                            tensor=dbounce.tensor,
                            offset=dbounce.offset,
                            ap=[[0, 64], [1, 512]],
                        ),
                    )
                    ot = outp.tile([64, 512], F32, tag="ot")
                    nc.vector.tensor_mul(ot[:], o[0:64, :], rbc[:])
                    nc.sync.dma_start(
                        out=out_d[hh * 64 : (hh + 1) * 64, i0 : i0 + 512], in_=ot[:]
                    )
                return

            # tail-optimized final normalization. Hop count is what matters:
            # den-copy + reciprocal back-to-back on DVE (no cross-engine sem
            # between them), broadcast matmul on a pre-warmed PE (six dummy
            # matmuls run during the last exp so the HAM clock stays up),
            # broadcast read-back on the idle ScalarE, muls on DVE, and the
            # two out-DMAs on separate queues.
            den_sb = [nrm.tile([1, 512], F32, tag="den", name=f"den{hp}") for hp in range(2)]
            recip = [nrm.tile([1, 512], F32, tag="recip", name=f"recip{hp}") for hp in range(2)]
            rbc = [nrm.tile([64, 512], F32, tag="rbc", name=f"rbc{hp}") for hp in range(2)]
            bc_ps = [ps_st.tile([128, 1024], F32, tag="st", name=f"bc{hp}") for hp in range(2)]
            ot = [outp.tile([64, 512], F32, tag="ot", name=f"ot{hp}") for hp in range(2)]
            for _ in range(6):
                nc.tensor.matmul(
                    bc_ps[0][64:128, 0:512], scr_sb[:, 0:64], scr_sb[:],
                    start=True, stop=True,
                )
            for hp in range(2):
                nc.vector.tensor_copy(den_sb[hp][:], o_ps[hp][64:65, :])
                nc.vector.reciprocal_approx_fast(out=recip[hp][:], in_=den_sb[hp][:])
                nc.tensor.matmul(
                    bc_ps[hp][0:64, 0:512], ones_sb[:], recip[hp][:],
                    start=True, stop=True,
                )
                nc.scalar.copy(rbc[hp][:], bc_ps[hp][0:64, 0:512])
            nc.vector.tensor_mul(ot[0][:], o_ps[0][0:64, :], rbc[0][:])
            nc.vector.tensor_mul(ot[1][:], o_ps[1][0:64, :], rbc[1][:])
            hh0 = 2 * p
            nc.sync.dma_start(
                out=out_d[hh0 * 64 : (hh0 + 1) * 64, i0 : i0 + 512], in_=ot[0][:]
            )
            nc.scalar.dma_start(
                out=out_d[(hh0 + 1) * 64 : (hh0 + 2) * 64, i0 : i0 + 512], in_=ot[1][:]
            )

        # prologue: just enough projection for the first scores (q0/k0 column
        # group 0); everything else is interleaved as fillers.
        # Constraints: vt[j] before av(j) of block (0,0); k[p] group m fully
        # projected+cast before st(4m) is EMITTED (iter 4m-1) of the first
        # block using pair p; q[p] group lc before block (p,lc). Projection
        # groups are sub-split so no single exp interval carries more than
        # ~0.43us of filler on top of st+av(+vt).
        def sched(*entries):
            m = {}
            for start, parts in entries:
                for i, part in enumerate(parts):
                    m.setdefault(start + i, []).append(part)
            return m

        SPLIT_211 = [[0, 1], [2], [3]]
        SPLIT_1111 = [[0], [1], [2], [3]]

        g_qk(0, 0, 0)()
        g_qk(0, 1, 0)()
        attn_block(0, 0, sched(
            *[(jb, [g_vt(jb)]) for jb in range(NJB)],
            (0, g_qk_split(0, 1, 1, SPLIT_211)),
            (3, g_qk_split(0, 1, 2, SPLIT_211)),
            (7, g_qk_split(0, 1, 3, SPLIT_1111)),
            (11, g_qk_split(0, 0, 1, SPLIT_1111)),
        ))
        attn_block(0, 1, sched((2, g_qk_split(0, 0, 2, SPLIT_1111))))
        attn_block(0, 2, sched((2, g_qk_split(0, 0, 3, SPLIT_1111))))
        attn_block(0, 3, sched(
            (1, g_qk_split(1, 1, 0, SPLIT_1111)),
            (8, g_qk_split(1, 0, 0, SPLIT_1111)),
        ))
        attn_block(1, 0, sched(
            (0, g_qk_split(1, 1, 1, SPLIT_211)),
            (3, g_qk_split(1, 1, 2, SPLIT_1111)),
            (7, g_qk_split(1, 1, 3, SPLIT_1111)),
            (11, g_qk_split(1, 0, 1, SPLIT_1111)),
        ))
        attn_block(1, 1, sched((2, g_qk_split(1, 0, 2, SPLIT_1111))))
        attn_block(1, 2, sched((2, g_qk_split(1, 0, 3, SPLIT_1111))))
        attn_block(1, 3, last=True)

    nc.finalize()
    return nc


def _get_nc():
    if "nc" not in _COMPILED:
        _COMPILED["nc"] = _build_nc()
    return _COMPILED["nc"]


def _prep_inputs(x, w_qkv):
    """Per-core input maps (host-side sharding)."""
    import ml_dtypes

    bf16 = ml_dtypes.bfloat16
    in_maps = []
    for c in range(N_CORES):
        b, g = c // 2, c % 2
        # x[b] [512, 2048] -> [p, lc, dc, l'] so every DMA descriptor is a
        # 4KB contiguous run
        xb = np.ascontiguousarray(
            x[b].reshape(4, 128, 4, 512).transpose(1, 2, 0, 3)
        ).astype(bf16)
        # w rows for this head group, transposed then laid out in 4 sections
        # s = 2*pair + (0 q | 1 k): [s, p, dc, o]; v separate — all DMAs are
        # fully contiguous
        wq_rows = w_qkv[256 * g : 256 * (g + 1), :]  # [256, 512]
        wk_rows = w_qkv[512 + 256 * g : 512 + 256 * (g + 1), :]
        wv_rows = w_qkv[1024 + 256 * g : 1024 + 256 * (g + 1), :]
        # section s covers head pair p = s//2, q (s%2==0) or k: 128 columns of
        # wT = rows 128*p..128*(p+1) of the q|k block
        secs = []
        for p in range(2):
            for rows in (wq_rows, wk_rows):
                sec = rows[128 * p : 128 * (p + 1), :].T  # [512(d), 128(o)]
                secs.append(sec.reshape(4, 128, 128).transpose(1, 0, 2))
        wqkT = np.ascontiguousarray(np.stack(secs, axis=0)).astype(bf16)
        wvT = np.ascontiguousarray(
            wv_rows.T.reshape(4, 128, 256).transpose(1, 0, 2)
        ).astype(bf16)
        in_maps.append({"x": xb, "wqkT": wqkT, "wvT": wvT})
    return in_maps


def kernel(x, w_qkv):
    global LAST_RESULTS
    from concourse.bass_utils import run_bass_kernel_spmd

    nc = _get_nc()
    in_maps = _prep_inputs(np.asarray(x), np.asarray(w_qkv))
    res = run_bass_kernel_spmd(
        nc, in_maps, core_ids=list(range(N_CORES)), trace=TRACE
    )
    LAST_RESULTS = res
    out = np.empty((B, D, L), dtype=np.float32)
    for c in range(N_CORES):
        b, g = c // 2, c % 2
        out[b, 256 * g : 256 * (g + 1), :] = res.results[c]["out"]
    return out


# revision 13
# speedup vs baseline: 1.0276x; 1.0273x over previous
"""Multi-head attention (qkv pointwise-conv projection + softmax attention)
on 8 Trainium2 NeuronCores.

Problem shapes (hardcoded):
    x:     [B=4, D=512, L=2048] f32
    w_qkv: [3*D=1536, D=512]    f32
    out:   [B, D, L]            f32

Sharding: 2 cores per batch element; each core owns 4 of the 8 heads
(tensor-parallel on the qkv output channels). Core c -> batch c//2,
head group c%2 (heads 4*(c%2) .. 4*(c%2)+3).

Per-core kernel (all in bf16 compute, f32 accumulate):
    Q/K proj:  q[o,l] = sum_d w[o,d] x[d,l]   (layout [head_dim, L])
    V proj  :  vT[l,o]                          (layout [L, head_dim])
               vT stored per head with a fused ones-column -> attn@[v|1]
               yields both the weighted values and the softmax denominator.
    scores  :  St[j,i] = sum_d k[d,j] q[d,i]  (two heads packed in the
               128-row PE array via row tiling: head0 partitions 0-63,
               head1 partitions 64-127)
    softmax :  exp on ScalarE (scale folded into the activation), no max
               subtraction (scores are O(1) by construction)
    attn@v  :  O[d(+den),i] accumulated over j blocks in PSUM
    norm    :  O[d,i] * broadcast(1/den[i])  (broadcast via K=1 matmul)

ScalarE is the bottleneck (128 exps x ~1.11us = 142us of 180us): the
schedule aims to keep the exp stream gapless. Startup: input DMAs go out
on three parallel DGE queues (SP + Act HWDGE, Pool SWDGE) with x chunk 0
split in halves and w_qkv split in per-(pair,q|k) sections so the first
scores land ~4us earlier. Fillers: pair-1 projections run inside pair-1
blocks (each block has ~4us of PE slack vs the exp cadence), k0 column
groups are front-loaded in block 0, vt is just-in-time. Tail: the final
block normalizes via ScalarE den-copies (idle after the last exp),
GpSimd does one of the two multiplies, and the two output DMAs use
separate queues.
"""

import os
import numpy as np

B, D, L, H = 4, 512, 2048, 8
HD = D // H  # 64
N_CORES = 8
SCALE = float(D) ** -0.5

# module-level knobs for test.py; harness uses defaults
TRACE = False
LAST_RESULTS = None

_COMPILED = {}


def _build_nc():
    from contextlib import ExitStack

    import concourse.bass as bass
    import concourse.mybir as mybir
    import concourse.tile as tile
    from concourse.bacc import Bacc

    F32 = mybir.dt.float32
    BF16 = mybir.dt.bfloat16
    Exp = mybir.ActivationFunctionType.Exp

    # Bacc (not plain Bass): its finalize() runs the legalization passes that
    # split multi-wait matmuls (walrus MM struct supports only 1 sync wait).
    nc = Bacc("TRN2", target_bir_lowering=False, debug=False)
    # host pre-permuted layouts -> fully contiguous DMA descriptors
    # x: [p, lc, dc, l'] where d = dc*128+p, l = lc*512+l'
    x_d = nc.dram_tensor("x", [128, 4, 4, 512], BF16, kind="ExternalInput")
    # wT split in 4 sections s = 2*pair + (0 q | 1 k): [s, p, dc, o]
    wqk_d = nc.dram_tensor("wqkT", [4, 128, 4, 128], BF16, kind="ExternalInput")
    wv_d = nc.dram_tensor("wvT", [128, 4, 256], BF16, kind="ExternalInput")
    out_d = nc.dram_tensor("out", [256, L], F32, kind="ExternalOutput")

    NJB = L // 128  # 16 key blocks
    NIC = L // 512  # 4 query chunks

    with ExitStack() as ctx:
        tc = ctx.enter_context(tile.TileContext(nc))
        const = ctx.enter_context(tc.tile_pool(name="const", bufs=1))
        qkp = ctx.enter_context(tc.tile_pool(name="qkp", bufs=1))
        vtp = ctx.enter_context(tc.tile_pool(name="vtp", bufs=1))
        sx = ctx.enter_context(tc.tile_pool(name="sx", bufs=6))
        nrm = ctx.enter_context(tc.tile_pool(name="nrm", bufs=4))
        outp = ctx.enter_context(tc.tile_pool(name="outp", bufs=4))
        drp = ctx.enter_context(tc.tile_pool(name="drp", bufs=4, space="DRAM"))
        ps_st = ctx.enter_context(tc.tile_pool(name="ps_st", bufs=2, space="PSUM"))
        ps_o = ctx.enter_context(tc.tile_pool(name="ps_o", bufs=4, space="PSUM"))

        # ---- PE warmup + load inputs ----
        # a few matmuls on zeros keep the PE busy while the first input DMAs
        # land, so the HAM clock gate starts opening (1.2 -> 2.4 GHz).
        scr_sb = const.tile([128, 512], BF16, tag="scr")
        nc.vector.memset(scr_sb[:], 0.0)
        warm_ps = ps_st.tile([128, 1024], F32, tag="st", name="warm")
        for _ in range(5):
            nc.tensor.matmul(warm_ps[:, 0:512], scr_sb[:, 0:128], scr_sb[:])

        wqk_sb = const.tile([128, 4, 4, 128], BF16, tag="wqk")
        wv_sb = const.tile([128, 4, 256], BF16, tag="wv")
        x_sb = const.tile([128, 4, 4, 512], BF16, tag="x")
        # All 16 DMA engines round-robin across every ACTIVE queue, so putting
        # streams on separate queues defeats prioritization (everything shares
        # the ~300GB/s aggregate). A single queue drains in FIFO order per
        # engine — so in-queue order IS the priority: the critical
        # first-projection set (wqk sections 0-1 + x chunk 0) first, then the
        # chunks in deadline order.
        nc.sync.dma_start(out=wqk_sb[:, 0, :, :], in_=wqk_d[0])
        nc.sync.dma_start(out=wqk_sb[:, 1, :, :], in_=wqk_d[1])
        nc.sync.dma_start(out=x_sb[:, 0, :, :], in_=x_d[:, 0, :, :])
        nc.sync.dma_start(out=x_sb[:, 1, :, :], in_=x_d[:, 1, :, :])
        nc.sync.dma_start(out=wv_sb[:], in_=wv_d[:])
        nc.sync.dma_start(out=x_sb[:, 2, :, :], in_=x_d[:, 2, :, :])
        nc.sync.dma_start(out=x_sb[:, 3, :, :], in_=x_d[:, 3, :, :])
        nc.sync.dma_start(out=wqk_sb[:, 2, :, :], in_=wqk_d[2])
        nc.sync.dma_start(out=wqk_sb[:, 3, :, :], in_=wqk_d[3])
        ones_sb = const.tile([1, 64], F32, tag="ones")
        nc.vector.memset(ones_sb[:], 1.0)

        q_sb = [qkp.tile([128, L], BF16, tag=f"q{p}", name=f"q{p}") for p in range(2)]
        k_sb = [qkp.tile([128, L], BF16, tag=f"k{p}", name=f"k{p}") for p in range(2)]
        vt_sb = [vtp.tile([128, 4, 65], BF16, tag=f"vt{jb}", name=f"vt{jb}") for jb in range(NJB)]

        # Projection groups run in 1-bank [128,512] PSUM tiles from the shared
        # "o" pool so they never contend with the exp-feeding st pipeline.
        def g_qk(p, qk, lc):
            # one 512-wide column group of the Q (qk=0) or K (qk=1)
            # projection for head-pair p
            def f():
                dst = q_sb[p] if qk == 0 else k_sb[p]
                ps = ps_o.tile([128, 512], F32, tag="o", name="projg")
                for dc in range(4):
                    nc.tensor.matmul(
                        ps[:],
                        wqk_sb[:, 2 * p + qk, dc, :],
                        x_sb[:, lc, dc, :],
                        start=(dc == 0),
                        stop=(dc == 3),
                    )
                nc.vector.tensor_copy(dst[:, lc * 512 : (lc + 1) * 512], ps[:])

            return f

        def g_qk_split(p, qk, lc, dcs):
            # g_qk sub-split into per-iteration pieces (each <=2 matmuls,
            # ~426ns) so filler load never spikes a single exp interval.
            # Returns a list of callables; schedule them on consecutive jbs.
            state = {}

            def part(dc_list):
                def f():
                    if 0 in dc_list:
                        state["ps"] = ps_o.tile(
                            [128, 512], F32, tag="o", name="projg"
                        )
                    for dc in dc_list:
                        nc.tensor.matmul(
                            state["ps"][:],
                            wqk_sb[:, 2 * p + qk, dc, :],
                            x_sb[:, lc, dc, :],
                            start=(dc == 0),
                            stop=(dc == 3),
                        )
                    if 3 in dc_list:
                        dst = q_sb[p] if qk == 0 else k_sb[p]
                        nc.vector.tensor_copy(
                            dst[:, lc * 512 : (lc + 1) * 512], state["ps"][:]
                        )

                return f

            return [part(dcl) for dcl in dcs]

        def g_vt(jb):
            def f():
                nc.vector.memset(vt_sb[jb][:, :, 64:65], 1.0)
                ps = ps_o.tile([128, 512], F32, tag="o", name="projv")
                for dc in range(4):
                    nc.tensor.matmul(
                        ps[:, 0:256],
                        x_sb[:, jb // 4, dc, (jb % 4) * 128 : (jb % 4 + 1) * 128],
                        wv_sb[:, dc, :],
                        start=(dc == 0),
                        stop=(dc == 3),
                    )
                nc.vector.tensor_copy(
                    vt_sb[jb][:, :, 0:64],
                    ps[:, 0:256].rearrange("par (h e) -> par h e", e=64),
                )

            return f

        def attn_block(p, ic, fillers=(), last=False):
            # scores+softmax+attn@v for head pair p, query chunk ic (512 wide)
            # fillers: {jb: [callables]} — projection groups interleaved into
            # the loop to fill PE slack without starving ScalarE
            # last: tail-optimized normalization (ScalarE den copies, GpSimd
            # mul, parallel out-DMA queues, PE broadcast)
            fillers = dict(fillers)
            i0 = ic * 512

            def st_mms(jb):
                # St[j, i] for both heads of the pair, row-packed in the PE
                st = ps_st.tile([128, 1024], F32, tag="st")
                for hp in range(2):
                    nc.tensor.matmul(
                        st[:, hp * 512 : (hp + 1) * 512],
                        k_sb[p][hp * 64 : (hp + 1) * 64, jb * 128 : (jb + 1) * 128],
                        q_sb[p][hp * 64 : (hp + 1) * 64, i0 : i0 + 512],
                        start=True,
                        stop=True,
                    )
                return st

            o_ps = [ps_o.tile([65, 512], F32, tag="o", name="o_acc") for _ in range(2)]
            st_cur = st_mms(0)
            for jb in range(NJB):
                se = sx.tile([128, 1024], BF16, tag="se")
                nc.scalar.activation(se[:], st_cur[:], Exp, scale=SCALE)
                if jb + 1 < NJB:
                    st_cur = st_mms(jb + 1)
                for f in fillers.get(jb, ()):
                    f()
                for hp in range(2):
                    nc.tensor.matmul(
                        o_ps[hp][:],
                        vt_sb[jb][:, 2 * p + hp, :],
                        se[:, hp * 512 : (hp + 1) * 512],
                        start=(jb == 0),
                        stop=(jb == NJB - 1),
                    )

            if not last:
                # normalize and write out. The o_ps ring slots gate the NEXT
                # block's projection fillers, so the PSUM accumulator must be
                # released ASAP: ONE [65,512] DVE copy (same cost as copying
                # just the den row — DVE time is free-dim-bound) moves o+den
                # to SBUF, then the whole chain (1/den on DVE, broadcast via
                # DRAM bounce, multiply) runs off SBUF without touching PSUM.
                for hp in range(2):
                    hh = 2 * p + hp
                    den_sb = nrm.tile([1, 512], F32, tag="den")
                    nc.vector.tensor_copy(den_sb[:], o_ps[hp][64:65, :])
                    oc = nrm.tile([64, 512], F32, tag="oc")
                    nc.vector.tensor_copy(oc[:], o_ps[hp][0:64, :])
                    recip = nrm.tile([1, 512], F32, tag="recip")
                    # NB: approx-recip reads garbage from PSUM on HW; SBUF in
                    # only, and DVE lanes are partition-aligned: in/out must
                    # start at the same partition (hence the den copy to p0).
                    nc.vector.reciprocal_approx_fast(out=recip[:], in_=den_sb[:])
                    rbc = nrm.tile([64, 512], F32, tag="rbc")
                    dbounce = drp.tile([1, 512], F32, tag="db", name="db")
                    nc.sync.dma_start(out=dbounce[:], in_=recip[:])
                    nc.sync.dma_start(
                        out=rbc[:],
                        in_=bass.AP(
                            tensor=dbounce.tensor,
                            offset=dbounce.offset,
                            ap=[[0, 64], [1, 512]],
                        ),
                    )
                    ot = outp.tile([64, 512], F32, tag="ot")
                    nc.vector.tensor_mul(ot[:], oc[0:64, :], rbc[:])
                    nc.sync.dma_start(
                        out=out_d[hh * 64 : (hh + 1) * 64, i0 : i0 + 512], in_=ot[:]
                    )
                return

            # tail-optimized final normalization. Hop count is what matters:
            # den-copy + reciprocal back-to-back on DVE (no cross-engine sem
            # between them), broadcast matmul on a pre-warmed PE (six dummy
            # matmuls run during the last exp so the HAM clock stays up),
            # broadcast read-back on the idle ScalarE, muls on DVE, and the
            # two out-DMAs on separate queues.
            den_sb = [nrm.tile([1, 512], F32, tag="den", name=f"den{hp}") for hp in range(2)]
            oc = [nrm.tile([64, 512], F32, tag="oc", name=f"oc{hp}") for hp in range(2)]
            recip = [nrm.tile([1, 512], F32, tag="recip", name=f"recip{hp}") for hp in range(2)]
            rbc = [nrm.tile([64, 512], F32, tag="rbc", name=f"rbc{hp}") for hp in range(2)]
            bc_ps = [ps_st.tile([128, 1024], F32, tag="st", name=f"bc{hp}") for hp in range(2)]
            ot = [outp.tile([64, 512], F32, tag="ot", name=f"ot{hp}") for hp in range(2)]
            for _ in range(4):
                nc.tensor.matmul(
                    bc_ps[0][64:128, 0:512], scr_sb[:, 0:64], scr_sb[:],
                    start=True, stop=True,
                )
            for hp in range(2):
                nc.vector.tensor_copy(den_sb[hp][:], o_ps[hp][64:65, :])
                nc.vector.reciprocal_approx_fast(out=recip[hp][:], in_=den_sb[hp][:])
                nc.vector.tensor_copy(oc[hp][:], o_ps[hp][0:64, :])
            for _ in range(3):
                nc.tensor.matmul(
                    bc_ps[0][64:128, 0:512], scr_sb[:, 0:64], scr_sb[:],
                    start=True, stop=True,
                )
            for hp in range(2):
                nc.tensor.matmul(
                    bc_ps[hp][0:64, 0:512], ones_sb[:], recip[hp][:],
                    start=True, stop=True,
                )
                nc.scalar.copy(rbc[hp][:], bc_ps[hp][0:64, 0:512])
            nc.vector.tensor_mul(ot[0][:], oc[0][0:64, :], rbc[0][:])
            nc.vector.tensor_mul(ot[1][:], oc[1][0:64, :], rbc[1][:])
            hh0 = 2 * p
            nc.sync.dma_start(
                out=out_d[hh0 * 64 : (hh0 + 1) * 64, i0 : i0 + 512], in_=ot[0][:]
            )
            nc.scalar.dma_start(
                out=out_d[(hh0 + 1) * 64 : (hh0 + 2) * 64, i0 : i0 + 512], in_=ot[1][:]
            )

        # prologue: just enough projection for the first scores (q0/k0 column
        # group 0); everything else is interleaved as fillers.
        # Constraints: vt[j] before av(j) of block (0,0); k[p] group m fully
        # projected+cast before st(4m) is EMITTED (iter 4m-1) of the first
        # block using pair p; q[p] group lc before block (p,lc). Projection
        # groups are sub-split so no single exp interval carries more than
        # ~0.43us of filler on top of st+av(+vt).
        def sched(*entries):
            m = {}
            for start, parts in entries:
                for i, part in enumerate(parts):
                    m.setdefault(start + i, []).append(part)
            return m

        SPLIT_211 = [[0, 1], [2], [3]]
        SPLIT_1111 = [[0], [1], [2], [3]]

        g_qk(0, 0, 0)()
        g_qk(0, 1, 0)()
        attn_block(0, 0, sched(
            *[(jb, [g_vt(jb)]) for jb in range(NJB)],
            (0, g_qk_split(0, 1, 1, SPLIT_211)),
            (3, g_qk_split(0, 1, 2, SPLIT_211)),
            (7, g_qk_split(0, 1, 3, SPLIT_1111)),
            (11, g_qk_split(0, 0, 1, SPLIT_1111)),
        ))
        attn_block(0, 1, sched((2, g_qk_split(0, 0, 2, SPLIT_1111))))
        attn_block(0, 2, sched((2, g_qk_split(0, 0, 3, SPLIT_1111))))
        attn_block(0, 3, sched(
            (1, g_qk_split(1, 1, 0, SPLIT_1111)),
            (8, g_qk_split(1, 0, 0, SPLIT_1111)),
        ))
        attn_block(1, 0, sched(
            (0, g_qk_split(1, 1, 1, SPLIT_211)),
            (3, g_qk_split(1, 1, 2, SPLIT_1111)),
            (7, g_qk_split(1, 1, 3, SPLIT_1111)),
            (11, g_qk_split(1, 0, 1, SPLIT_1111)),
        ))
        attn_block(1, 1, sched((2, g_qk_split(1, 0, 2, SPLIT_1111))))
        attn_block(1, 2, sched((2, g_qk_split(1, 0, 3, SPLIT_1111))))
        attn_block(1, 3, last=True)

    nc.finalize()
    return nc


def _get_nc():
    if "nc" not in _COMPILED:
        _COMPILED["nc"] = _build_nc()
    return _COMPILED["nc"]


def _prep_inputs(x, w_qkv):
    """Per-core input maps (host-side sharding)."""
    import ml_dtypes

    bf16 = ml_dtypes.bfloat16
    in_maps = []
    for c in range(N_CORES):
        b, g = c // 2, c % 2
        # x[b] [512, 2048] -> [p, lc, dc, l'] so every DMA descriptor is a
        # 4KB contiguous run
        xb = np.ascontiguousarray(
            x[b].reshape(4, 128, 4, 512).transpose(1, 2, 0, 3)
        ).astype(bf16)
        # w rows for this head group, transposed then laid out in 4 sections
        # s = 2*pair + (0 q | 1 k): [s, p, dc, o]; v separate — all DMAs are
        # fully contiguous
        wq_rows = w_qkv[256 * g : 256 * (g + 1), :]  # [256, 512]
        wk_rows = w_qkv[512 + 256 * g : 512 + 256 * (g + 1), :]
        wv_rows = w_qkv[1024 + 256 * g : 1024 + 256 * (g + 1), :]
        # section s covers head pair p = s//2, q (s%2==0) or k: 128 columns of
        # wT = rows 128*p..128*(p+1) of the q|k block
        secs = []
        for p in range(2):
            for rows in (wq_rows, wk_rows):
                sec = rows[128 * p : 128 * (p + 1), :].T  # [512(d), 128(o)]
                secs.append(sec.reshape(4, 128, 128).transpose(1, 0, 2))
        wqkT = np.ascontiguousarray(np.stack(secs, axis=0)).astype(bf16)
        wvT = np.ascontiguousarray(
            wv_rows.T.reshape(4, 128, 256).transpose(1, 0, 2)
        ).astype(bf16)
        in_maps.append({"x": xb, "wqkT": wqkT, "wvT": wvT})
    return in_maps


def kernel(x, w_qkv):
    global LAST_RESULTS
    from concourse.bass_utils import run_bass_kernel_spmd

    nc = _get_nc()
    in_maps = _prep_inputs(np.asarray(x), np.asarray(w_qkv))
    res = run_bass_kernel_spmd(
        nc, in_maps, core_ids=list(range(N_CORES)), trace=TRACE
    )
    LAST_RESULTS = res
    out = np.empty((B, D, L), dtype=np.float32)
    for c in range(N_CORES):
        b, g = c // 2, c % 2
        out[b, 256 * g : 256 * (g + 1), :] = res.results[c]["out"]
    return out
